# revision 19
# baseline (speedup 1.0000x reference)
"""EventSNNFlowNetLite Bass kernel (per-core program) + host-side packing.

Sharding: 8 cores = 4 images x 2 vertical halves; each core computes its
half with redundant halo rows (no inter-core communication).

Conv = PSUM-accumulated matmuls with strided APs; decoder convs on up2(X)
are 4-phase 2x2 stencils on the half-res input. The two column taps of the
d2/d1 stencils are K-stacked: a col-shifted twin copy of the source spikes
(written by a twin fused-spike stt) doubles K so each phase needs 2
accumulating matmuls instead of 4. LIF membranes are stored NEGATED
(M = spike - mem, exact in fp32) so decay+accumulate, spike+skip-add, and
reset are one scalar_tensor_tensor each. d1 keeps its four 32-row bands on
the four partition strips of one [128, 36, 256] mem tile, so its matmuls
run 4-col-tile concurrent and all its DVE ops are 128 partitions wide.
d1 spikes live in a bf16 [128, 36, 258] scratch (0/1 exact) consumed by
the final flow conv with bf16 weights (only inexactness: ~0.2% rel on the
last layer; rel L2 vs reference 2.0e-3). e1/d2 membranes are banded across
partition strips (m1b/md2b) so their update+reset stts run 128 wide; the
temporal means are kept as exact integer spike SUMS with the 1/(t+1) scale
folded into 8 pre-scaled copies of the skip weights.

Per-core bottom-edge phantom rows (below-image halo on bottom-half cores)
are zeroed via a per-core 0/1 mask input so decoder edge rows see conv
zero-padding; top/bottom halo rows above the image auto-zero through the
zero-filled x im2col.
"""
import ml_dtypes
import numpy as np
import concourse.bass as bass
import concourse.mybir as mybir
from concourse.tile import TileContext

F32 = mybir.dt.float32
T_STEPS = 8
ALU = mybir.AluOpType

DBG_NAMES = ('s1', 's2', 's3', 'd3', 'd2', 'me1', 'me2', 'm1', 'm2')


# ------------------------------------------------------------- host packing

def phase_stencils(w):
    """w: [O, I, 3, 3] -> dict[(pr, pc, a, b)] = [I, O] combined stencils."""
    rows = {(0, 0): [0], (0, 1): [1, 2], (1, 0): [0, 1], (1, 1): [2]}
    out = {}
    for pr in range(2):
        for pc in range(2):
            for a in range(2):
                for b in range(2):
                    acc = np.zeros(w.shape[:2], np.float32)
                    for ky in rows[(pr, a)]:
                        for kx in rows[(pc, b)]:
                            acc = acc + w[:, :, ky, kx]
                    out[(pr, pc, a, b)] = np.ascontiguousarray(acc.T)
    return out


def pack_weights(inputs):
    w = {}
    w['w_e1m'] = np.ascontiguousarray(
        np.asarray(inputs['w_e1']).reshape(32, 50).T).astype(np.float32)  # [50,32]
    for nm, key in (('w_e2t', 'w_e2'), ('w_e3t', 'w_e3')):
        ww = np.asarray(inputs[key])
        I = ww.shape[1]
        t = np.stack([np.ascontiguousarray(ww[:, :, ky, kx].T)
                      for ky in range(3) for kx in range(3)])  # [9, I, O]
        w[nm] = np.ascontiguousarray(t.transpose(1, 0, 2)).reshape(I, -1)
    S = phase_stencils(np.asarray(inputs['w_d3']))
    t = np.stack([S[(pr, pc, a, b)] for pr in range(2) for pc in range(2)
                  for a in range(2) for b in range(2)])  # [16, 128, 64]
    w['w_d3s'] = np.ascontiguousarray(t.transpose(1, 0, 2)).reshape(128, -1)
    # K-stacked pair weights: [lowhalf; highhalf] partitions contract the
    # (bb=?) tap halves in one matmul.
    for nm, key, I, lowbb in (('w_d2s', 'w_d2', 64, 1), ('w_d1s', 'w_d1', 32, 0)):
        S = phase_stencils(np.asarray(inputs[key]))
        cols = []
        for pr in range(2):
            for pc in range(2):
                for ab in range(2):
                    lo = S[(pr, pc, ab, lowbb)]      # [I, O]
                    hi = S[(pr, pc, ab, 1 - lowbb)]
                    cols.append(np.concatenate([lo, hi], axis=0))  # [2I, O]
        w[nm] = np.ascontiguousarray(
            np.stack(cols).transpose(1, 0, 2)).reshape(2 * I, -1)
    sk2 = np.asarray(inputs['w_skip2'])[:, :, 0, 0].T.astype(np.float64)
    sk1 = np.asarray(inputs['w_skip1'])[:, :, 0, 0].T.astype(np.float64)
    w['w_sk2'] = np.concatenate(
        [(sk2 / (t + 1)).astype(np.float32) for t in range(8)], axis=1)
    w['w_sk1'] = np.concatenate(
        [(sk1 / (t + 1)).astype(np.float32) for t in range(8)], axis=1)
    wf = np.asarray(inputs['w_flow']) * 16.0
    t = np.stack([np.ascontiguousarray(wf[:, :, ky, kx].T)
                  for ky in range(3) for kx in range(3)])  # [9, 32, 2]
    w['w_flt'] = np.ascontiguousarray(t.transpose(1, 0, 2)).reshape(32, 18).astype(np.float32)
    wf4 = np.zeros((128, 36), np.float32)
    wf4[:, 0:18] = np.tile(w['w_flt'], (4, 1))
    wf4[32:64, 18:36] = w['w_flt']
    wf4[96:128, 18:36] = w['w_flt']
    w['w_flt4'] = wf4.astype(ml_dtypes.bfloat16)
    return w


def pack_x_core(x_img, a):
    """x_img [T,2,256,256] -> [T, 2(chunks), 50, 40*128] fp32 im2col halves."""
    T = x_img.shape[0]
    xp = np.zeros((T, 2, 153, 260), np.float32)
    r0, r1 = a - 16, a + 137
    sr0, sr1 = max(r0, 0), min(r1, 256)
    xp[:, :, sr0 - r0:sr1 - r0, 2:258] = np.asarray(x_img)[:, :, sr0:sr1, :]
    full = np.empty((T, 50, 75, 128), np.float32)
    for c in range(2):
        for ky in range(5):
            for kx in range(5):
                full[:, c * 25 + ky * 5 + kx] = xp[:, c, ky:ky + 149:2, kx:kx + 255:2]
    out = np.zeros((T, 2, 50, 40, 128), np.float32)
    out[:, 0] = full[:, :, 0:40]
    out[:, 1, :, 0:35] = full[:, :, 40:75]
    return out.reshape(T, 2, 50, 40 * 128)


# ------------------------------------------------------------- device kernel

def build_kernel(repeats=1, debug=False):
    nc = bass.Bass("TRN2", target_bir_lowering=False, debug=False, num_devices=8)
    xd = nc.dram_tensor("x_e1", [T_STEPS, 2, 50, 40 * 128], F32, kind="ExternalInput").ap()
    wd = {}
    for nm, shape in (
        ('w_e1m', [50, 32]), ('w_e2t', [32, 9 * 64]), ('w_e3t', [64, 9 * 128]),
        ('w_d3s', [128, 16 * 64]), ('w_d2s', [128, 8 * 32]), ('w_d1s', [64, 8 * 32]),
        ('w_sk2', [64, 512]), ('w_sk1', [32, 256]), ('w_flt', [32, 18]),
    ):
        wd[nm] = nc.dram_tensor(nm, shape, F32, kind="ExternalInput").ap()
    flow_d = nc.dram_tensor("flow", [128, 2, 256], F32, kind="ExternalOutput").ap()
    # per-core bottom-edge mask: 1.0 on cores whose below-half phantom rows are
    # valid halo (top-half cores), 0.0 where they fall below the image
    # (bottom-half cores) and must act as conv zero-pad.
    mask_d = nc.dram_tensor("edge_mask", [128, 258], F32, kind="ExternalInput").ap()
    maskb_d = nc.dram_tensor("edge_mask_bf", [128, 258], mybir.dt.bfloat16,
                             kind="ExternalInput").ap()
    wfb_d = nc.dram_tensor("w_flt4", [128, 36], mybir.dt.bfloat16,
                           kind="ExternalInput").ap()
    dbg_d = {}
    if debug:
        for nm, shape in (('s1', [32, 75 * 130]), ('s2', [64, 37 * 66]),
                          ('s3', [128, 18 * 34]), ('d3', [64, 34 * 66]),
                          ('d2', [32, 66 * 130]), ('me1', [32, 66 * 130]),
                          ('me2', [64, 34 * 66]), ('m1', [32, 75 * 130]),
                          ('m2', [64, 37 * 66])):
            dbg_d[nm] = nc.dram_tensor(f"dbg_{nm}", shape, F32, kind="ExternalOutput").ap()

    with TileContext(nc) as tc:
        with tc.tile_pool(name="big", bufs=1) as sp, \
             tc.tile_pool(name="psum", bufs=8, space="PSUM") as pp:

            # ---- mega tiles (partition-slot packed)
            # tA: s1(0:32) | m1(32:64) | s2+me2+m2+d3 (64:128)
            # tB: md3(64:128, 0:2244) | xsl+w_e1m (32:82, 2244:7396)
            # tC: me1+w_sk1 (0:32) | md2(32:64) | d2+w_d1s (64:96)
            # tD: md1 [128, 36, 256] fp32 (4 d1 row-bands on 4 strips)
            # tE: d1scr [128, 36, 258] bf16 (d1 spikes, flow conv rhs)
            tA = sp.tile([128, 9750], F32, name="tA")
            tB = sp.tile([128, 7396], F32, name="tB")
            tC = sp.tile([128, 9092], F32, name="tC")
            tD = sp.tile([128, 9216], F32, name="tD")
            tE = sp.tile([128, 9288], mybir.dt.bfloat16, name="tE")
            tH = sp.tile([128, 2248], F32, name="tH")
            tW = sp.tile([128, 2804], F32, name="tW")
            tM = sp.tile([128, 258], mybir.dt.bfloat16, name="tM")
            tN1 = sp.tile([128, 3120], F32, name="tN1")  # banded m1 [128,24,130]
            tN2 = sp.tile([128, 2210], F32, name="tN2")  # banded md2 [128,17,130]
            for t_ in (tA, tB, tC, tD, tE, tH, tW, tM, tN1, tN2):
                nc.vector.memset(t_[:], 0.0)

            def view(tile, pb, pn, o0, R, W):
                return tile[pb:pb + pn, o0:o0 + R * W].rearrange(
                    "p (r w) -> p r w", w=W)

            s1 = view(tA, 0, 32, 0, 75, 130)
            m1b = view(tN1, 0, 128, 0, 24, 130)
            s2 = view(tA, 64, 64, 0, 37, 66)
            me2 = view(tA, 64, 64, 2442, 34, 66)
            m2 = view(tA, 64, 64, 4686, 37, 66)
            md3 = view(tA, 64, 64, 7128, 34, 66)
            d3 = view(tB, 64, 64, 0, 34, 66)
            d3s = view(tB, 0, 64, 0, 34, 66)   # col-shifted twin (bb=1 taps)
            d3k = view(tB, 0, 128, 0, 34, 66)  # stacked K=128 rhs for d2
            xsl = tB[64:114, 2244:7364]   # [50, 5120] x chunk slot (ch 0)
            xslB = tB[0:50, 2244:7364]    # second slot (ch 1) for DMA overlap
            me1 = view(tC, 0, 32, 0, 66, 130)
            md2b = view(tN2, 0, 128, 0, 17, 130)
            d2 = view(tC, 64, 32, 0, 66, 130)
            d2s = view(tC, 96, 32, 0, 66, 130)  # col-shifted twin
            d2k = view(tC, 64, 64, 0, 66, 130)  # stacked K=64 rhs for d1
            md1 = view(tD, 0, 128, 0, 36, 256)
            d1scr = view(tE, 0, 128, 0, 36, 258)
            s3 = view(tH, 0, 128, 0, 18, 34)
            m3 = view(tH, 0, 128, 612, 18, 34)
            w_d3s = tH[0:128, 1224:1224 + 1024]
            floscr = tW[0:2, 0:1024].rearrange("p (r w) -> p r w", w=256)  # [2,4,256]
            wsl = {
                'w_e2t': tW[0:32, 1024:1024 + 576],
                'w_sk1': tC[0:32, 8580:8836],
                'w_d2s': tW[0:128, 2548:2804],
                'w_e1m': tB[64:114, 7364:7396],
                'w_e1mB': tB[0:50, 7364:7396],
                'w_e3t': tW[64:128, 32:32 + 1152],
                'w_sk2': tW[64:128, 1760:1760 + 512],
                'w_d1s': tC[64:128, 8580:8836],
            }
            w_flt4 = None  # bf16 flow weights, one copy per strip; own tile
            tFw = sp.tile([128, 36], mybir.dt.bfloat16, name="tFw")
            edge_mask = tW[0:128, 2272:2530]
            mask_bf = tM[0:128, 0:258]
            nc.gpsimd.dma_start(out=edge_mask, in_=mask_d[:])
            nc.gpsimd.dma_start(out=mask_bf, in_=maskb_d[:])
            nc.gpsimd.dma_start(out=tFw[:], in_=wfb_d[:])
            nc.gpsimd.dma_start(out=w_d3s, in_=wd['w_d3s'][:])
            for nm, ap in wsl.items():
                nc.gpsimd.dma_start(out=ap, in_=wd['w_e1m' if nm == 'w_e1mB'
                                                  else nm][:])

            def lif(mem_ap, psum_ap, s_ap, write_mem=True):
                nc.vector.scalar_tensor_tensor(
                    out=mem_ap, in0=mem_ap, scalar=0.5, in1=psum_ap,
                    op0=ALU.mult, op1=ALU.add)
                nc.vector.tensor_scalar(
                    out=s_ap, in0=mem_ap, scalar1=1.0, scalar2=None, op0=ALU.is_gt)
                if write_mem:
                    nc.vector.tensor_tensor(
                        out=mem_ap, in0=mem_ap, in1=s_ap, op=ALU.subtract)

            def psum_tile(base, n):
                ps = pp.tile([128, 512], F32, name="ps", tag="ps")
                return ps[base:base + n, :]

            taps9 = [(ky, kx) for ky in range(3) for kx in range(3)]

            def enc_layer(src, dst, mem, wt, R_out, IC_out, C_out, trow, last_mem):
                # col-tiled rounds: 128//C_out concurrent blocks per psum bank
                nr_max = 512 // IC_out
                ng = 128 // C_out
                blocks = []
                q0 = 0
                while q0 < R_out:
                    blocks.append((q0, min(nr_max, R_out - q0)))
                    q0 += nr_max
                for rs in range(0, len(blocks), ng):
                    rnd = blocks[rs:rs + ng]
                    ps = pp.tile([128, 512], F32, name="ps", tag="ps")
                    views = []
                    for j, (q0, nr) in enumerate(rnd):
                        views.append(ps[j * C_out:(j + 1) * C_out, :nr * IC_out]
                                     .rearrange("p (r w) -> p r w", w=IC_out))
                    for i, (ky, kx) in enumerate(taps9):
                        for j, (q0, nr) in enumerate(rnd):
                            rhs = src[:, 2 * q0 + ky: 2 * q0 + ky + 2 * (nr - 1) + 1: 2,
                                      kx: kx + 2 * (IC_out - 1) + 1: 2]
                            nc.tensor.matmul(views[j], wt[:, i * C_out:(i + 1) * C_out],
                                             rhs, start=(i == 0), stop=(i == 8),
                                             tile_position=(trow, j * C_out))
                    for j, (q0, nr) in enumerate(rnd):
                        nc.vector.scalar_tensor_tensor(
                            out=mem[:, q0:q0 + nr, 1:1 + IC_out],
                            in0=mem[:, q0:q0 + nr, 1:1 + IC_out], scalar=0.5,
                            in1=views[j], op0=ALU.mult, op1=ALU.add)
                    uq0 = rnd[0][0]
                    uqn = rnd[-1][0] + rnd[-1][1] - uq0
                    nc.vector.tensor_scalar(
                        out=dst[:, uq0:uq0 + uqn, 1:1 + IC_out],
                        in0=mem[:, uq0:uq0 + uqn, 1:1 + IC_out], scalar1=1.0,
                        scalar2=None, op0=ALU.is_gt)
                    if not last_mem:
                        nc.vector.tensor_tensor(
                            out=mem[:, uq0:uq0 + uqn, 1:1 + IC_out],
                            in0=mem[:, uq0:uq0 + uqn, 1:1 + IC_out],
                            in1=dst[:, uq0:uq0 + uqn, 1:1 + IC_out], op=ALU.subtract)

            def dec_layer(src, dst, mem, wt, n_k, half, C_out, trow, skip_wt=None,
                          skip_src=None, skip_trow=0, last_mem=False,
                          kpair=False, shadow=None):
                """phase conv on up2(src); dst/mem row i0+2k; src row k+a.

                mem is stored NEGATED between steps (M = spike - mem, exact in
                fp32), so the update is M' = -0.5*M + cur, the spike+skip-add
                fuse into one stt from PSUM, and the reset is one stt:
                M = (mem' > 1) - mem'.
                """
                nr_max = 512 // half
                ng = 128 // C_out
                for pr in range(2):
                    i0 = 1 - pr
                    for pc in range(2):
                        blocks = []
                        k0 = 0
                        while k0 < n_k:
                            blocks.append((k0, min(nr_max, n_k - k0)))
                            k0 += nr_max
                        for rs in range(0, len(blocks), ng):
                            rnd = blocks[rs:rs + ng]
                            ps = pp.tile([128, 512], F32, name="ps", tag="ps")
                            views = [ps[j * C_out:(j + 1) * C_out, :nr * half]
                                     .rearrange("p (r w) -> p r w", w=half)
                                     for j, (k0, nr) in enumerate(rnd)]
                            if kpair:
                                for ab in range(2):
                                    wslice = wt[:, ((pr * 2 + pc) * 2 + ab) * C_out:
                                                ((pr * 2 + pc) * 2 + ab + 1) * C_out]
                                    for j, (k0, nr) in enumerate(rnd):
                                        rhs = src[:, k0 + ab: k0 + ab + nr,
                                                  pc: pc + half]
                                        nc.tensor.matmul(views[j], wslice, rhs,
                                                         start=(ab == 0), stop=(ab == 1),
                                                         tile_position=(trow, j * C_out))
                            else:
                              for idx, (a, b) in enumerate(
                                      ((0, 0), (0, 1), (1, 0), (1, 1))):
                                wslice = wt[:, (((pr * 2 + pc) * 2 + a) * 2 + b) * C_out:
                                            (((pr * 2 + pc) * 2 + a) * 2 + b + 1) * C_out]
                                for j, (k0, nr) in enumerate(rnd):
                                    rhs = src[:, k0 + a: k0 + a + nr,
                                              b + pc: b + pc + half]
                                    nc.tensor.matmul(views[j], wslice, rhs,
                                                     start=(idx == 0), stop=(idx == 3),
                                                     tile_position=(trow, j * C_out))
                            for j, (k0, nr) in enumerate(rnd):
                                rows = slice(i0 + 2 * k0, i0 + 2 * (k0 + nr - 1) + 1, 2)
                                cols = slice(1 + pc, 1 + pc + 2 * (half - 1) + 1, 2)
                                nc.vector.scalar_tensor_tensor(
                                    out=mem[:, rows, cols], in0=mem[:, rows, cols],
                                    scalar=-0.5, in1=views[j],
                                    op0=ALU.mult, op1=ALU.add)
                        ucols = slice(1 + pc, 1 + pc + 2 * (half - 1) + 1, 2)
                        urows = slice(i0, i0 + 2 * (n_k - 1) + 1, 2)
                        if skip_wt is not None:
                            # fused spike + skip-add per block round
                            for rs in range(0, len(blocks), ng):
                                rnd = blocks[rs:rs + ng]
                                ps2 = pp.tile([128, 512], F32, name="ps2", tag="ps")
                                for j, (k0, nr) in enumerate(rnd):
                                    v2 = ps2[j * C_out:(j + 1) * C_out, :nr * half]\
                                        .rearrange("p (r w) -> p r w", w=half)
                                    rows = slice(i0 + 2 * k0,
                                                 i0 + 2 * (k0 + nr - 1) + 1, 2)
                                    nc.tensor.matmul(v2, skip_wt,
                                                     skip_src[:, rows, ucols],
                                                     start=True, stop=True,
                                                     tile_position=(skip_trow, j * C_out))
                                    nc.vector.scalar_tensor_tensor(
                                        out=dst[:, rows, ucols],
                                        in0=mem[:, rows, ucols], scalar=1.0,
                                        in1=v2, op0=ALU.is_gt, op1=ALU.add)
                                    if shadow is not None:
                                        ucols2 = slice(ucols.start - 1,
                                                       ucols.stop - 1, 2)
                                        nc.vector.scalar_tensor_tensor(
                                            out=shadow[:, rows, ucols2],
                                            in0=mem[:, rows, ucols], scalar=1.0,
                                            in1=v2, op0=ALU.is_gt, op1=ALU.add)
                        else:
                            nc.vector.tensor_scalar(
                                out=dst[:, urows, ucols], in0=mem[:, urows, ucols],
                                scalar1=1.0, scalar2=None, op0=ALU.is_gt)
                        if not last_mem:
                            # M = (mem' > 1) - mem'  (negated store)
                            nc.vector.scalar_tensor_tensor(
                                out=mem[:, urows, ucols], in0=mem[:, urows, ucols],
                                scalar=1.0, in1=mem[:, urows, ucols],
                                op0=ALU.is_gt, op1=ALU.subtract)

            def dec_d2(src, dst, memb, wt, skip_wt, skip_src,
                       last_mem=False, shadow=None):
                """d2 specialization: n_k=33, half=64, C_out=32, kpair taps.

                memb [128, 17, 130]: strip j holds d2 buffer rows
                [16j, 16j+16) at local rows 0..15; stragglers: buffer row 64
                at strip 0 local 16, row 65 at strip 1 local 16. Mem update
                and reset are single 128-wide stt ops per phase.
                """
                for pr in range(2):
                    i0 = 1 - pr
                    for pc in range(2):
                        cols = slice(1 + pc, 1 + pc + 2 * 63 + 1, 2)
                        cols2 = slice(pc, pc + 2 * 63 + 1, 2)
                        lrows = slice(i0, i0 + 2 * 7 + 1, 2)
                        ps = pp.tile([128, 512], F32, name="ps", tag="ps")
                        views = [ps[32 * j:32 * j + 32, :512].rearrange(
                            "p (r w) -> p r w", w=64) for j in range(4)]
                        pss = pp.tile([128, 512], F32, name="pss", tag="ps")
                        vs = pss[0:32, :64].rearrange("p (r w) -> p r w", w=64)
                        for ab in range(2):
                            wslice = wt[:, ((pr * 2 + pc) * 2 + ab) * 32:
                                        ((pr * 2 + pc) * 2 + ab + 1) * 32]
                            for j in range(4):
                                rhs = src[:, 8 * j + ab: 8 * j + ab + 8,
                                          pc: pc + 64]
                                nc.tensor.matmul(views[j], wslice, rhs,
                                                 start=(ab == 0), stop=(ab == 1),
                                                 tile_position=(0, 32 * j))
                            nc.tensor.matmul(vs, wslice,
                                             src[:, 32 + ab: 33 + ab, pc: pc + 64],
                                             start=(ab == 0), stop=(ab == 1),
                                             tile_position=(0, 0))
                        nc.vector.scalar_tensor_tensor(
                            out=memb[:, lrows, cols], in0=memb[:, lrows, cols],
                            scalar=-0.5,
                            in1=ps[0:128, :512].rearrange("p (r w) -> p r w", w=64),
                            op0=ALU.mult, op1=ALU.add)
                        nc.vector.scalar_tensor_tensor(
                            out=memb[32 * i0:32 * i0 + 32, 16:17, cols],
                            in0=memb[32 * i0:32 * i0 + 32, 16:17, cols],
                            scalar=-0.5, in1=vs, op0=ALU.mult, op1=ALU.add)
                        # fused spike + skip-add (+ shadow twin) per block
                        ps2 = pp.tile([128, 512], F32, name="ps2", tag="ps")
                        for j in range(4):
                            v2 = ps2[32 * j:32 * j + 32, :512].rearrange(
                                "p (r w) -> p r w", w=64)
                            grows = slice(16 * j + i0, 16 * j + i0 + 2 * 7 + 1, 2)
                            nc.tensor.matmul(v2, skip_wt,
                                             skip_src[:, grows, cols],
                                             start=True, stop=True,
                                             tile_position=(0, 32 * j))
                            nc.vector.scalar_tensor_tensor(
                                out=dst[:, grows, cols],
                                in0=memb[32 * j:32 * j + 32, lrows, cols],
                                scalar=1.0, in1=v2, op0=ALU.is_gt, op1=ALU.add)
                            if shadow is not None:
                                nc.vector.scalar_tensor_tensor(
                                    out=shadow[:, grows, cols2],
                                    in0=memb[32 * j:32 * j + 32, lrows, cols],
                                    scalar=1.0, in1=v2,
                                    op0=ALU.is_gt, op1=ALU.add)
                        ps2s = pp.tile([128, 512], F32, name="ps2s", tag="ps")
                        v2s = ps2s[0:32, :64].rearrange("p (r w) -> p r w", w=64)
                        nc.tensor.matmul(v2s, skip_wt,
                                         skip_src[:, 64 + i0:65 + i0, cols],
                                         start=True, stop=True,
                                         tile_position=(0, 0))
                        nc.vector.scalar_tensor_tensor(
                            out=dst[:, 64 + i0:65 + i0, cols],
                            in0=memb[32 * i0:32 * i0 + 32, 16:17, cols],
                            scalar=1.0, in1=v2s, op0=ALU.is_gt, op1=ALU.add)
                        if shadow is not None:
                            nc.vector.scalar_tensor_tensor(
                                out=shadow[:, 64 + i0:65 + i0, cols2],
                                in0=memb[32 * i0:32 * i0 + 32, 16:17, cols],
                                scalar=1.0, in1=v2s, op0=ALU.is_gt, op1=ALU.add)
                        if not last_mem:
                            nc.vector.scalar_tensor_tensor(
                                out=memb[:, lrows, cols], in0=memb[:, lrows, cols],
                                scalar=1.0, in1=memb[:, lrows, cols],
                                op0=ALU.is_gt, op1=ALU.subtract)
                            nc.vector.scalar_tensor_tensor(
                                out=memb[32 * i0:32 * i0 + 32, 16:17, cols],
                                in0=memb[32 * i0:32 * i0 + 32, 16:17, cols],
                                scalar=1.0, in1=memb[32 * i0:32 * i0 + 32, 16:17, cols],
                                op0=ALU.is_gt, op1=ALU.subtract)

            # band k-ranges for d1: band g covers phase rows k in [16g, kend)
            d1_kend = [16, 32, 48, 65]

            for rep in range(repeats):
                for t in range(T_STEPS):
                    last = (t == T_STEPS - 1) and (rep == repeats - 1)
                    # ---- e1 in two x-chunks (rows 0-39, 40-74); two SBUF
                    # slots so the ch-1 DMA overlaps ch-0 compute.
                    nc.sync.dma_start(out=xsl, in_=xd[t, 0])
                    nc.sync.dma_start(out=xslB, in_=xd[t, 1])
                    R = 0  # global e1 round index -> m1b local row base 4R
                    for ch, (cr0, crn) in enumerate(((0, 40), (40, 35))):
                        slot, wkey, trow = ((xsl, 'w_e1m', 64),
                                            (xslB, 'w_e1mB', 0))[ch]
                        xv = slot.rearrange("p (r w) -> p r w", w=128)
                        blocks = []
                        r0 = 0
                        while r0 < crn:
                            blocks.append((r0, min(4, crn - r0)))
                            r0 += 4
                        for rs in range(0, len(blocks), 4):
                            rnd = blocks[rs:rs + 4]
                            nj = len(rnd)
                            nr = rnd[0][1]  # uniform within a round
                            ps = pp.tile([128, 512], F32, name="ps", tag="ps")
                            views = [ps[32 * j:32 * j + 32, :n_ * 128].rearrange(
                                "p (r w) -> p r w", w=128)
                                for j, (r0, n_) in enumerate(rnd)]
                            for j, (r0, n_) in enumerate(rnd):
                                nc.tensor.matmul(views[j], wsl[wkey],
                                                 xv[:, r0:r0 + n_, :],
                                                 start=True, stop=True,
                                                 tile_position=(trow, 32 * j))
                            lb = 4 * R
                            nc.vector.scalar_tensor_tensor(
                                out=m1b[0:32 * nj, lb:lb + nr, 1:129],
                                in0=m1b[0:32 * nj, lb:lb + nr, 1:129],
                                scalar=-0.5,
                                in1=ps[0:32 * nj, :nr * 128].rearrange(
                                    "p (r w) -> p r w", w=128),
                                op0=ALU.mult, op1=ALU.add)
                            for j, (r0, n_) in enumerate(rnd):
                                gr = cr0 + r0
                                nc.vector.tensor_scalar(
                                    out=s1[:, gr:gr + n_, 1:129],
                                    in0=m1b[32 * j:32 * j + 32, lb:lb + n_, 1:129],
                                    scalar1=1.0, scalar2=None, op0=ALU.is_gt)
                            if not last:
                                nc.vector.scalar_tensor_tensor(
                                    out=m1b[0:32 * nj, lb:lb + nr, 1:129],
                                    in0=m1b[0:32 * nj, lb:lb + nr, 1:129],
                                    scalar=1.0,
                                    in1=m1b[0:32 * nj, lb:lb + nr, 1:129],
                                    op0=ALU.is_gt, op1=ALU.subtract)
                            R += 1

                    # ---- e2: s1 -> s2 (K=32, psum col 64)
                    enc_layer(s1, s2, m2, wsl['w_e2t'], 37, 64, 64, 0, last)
                    # ---- e3: s2 -> s3 (K=64 row base 64, psum col 0)
                    enc_layer(s2, s3, m3, wsl['w_e3t'], 18, 32, 128, 64, last)
                    # zero phantom s3 row (buffer row 17 = below-image on
                    # bottom cores) so d3's edge rows see conv zero-pad.
                    nc.vector.tensor_tensor(
                        out=s3[:, 17:18, :], in0=s3[:, 17:18, :],
                        in1=edge_mask[0:128, 0:34].rearrange(
                            "p (r w) -> p r w", w=34),
                        op=ALU.mult)

                    # ---- temporal spike sums (means folded into per-t
                    # pre-scaled skip weights; sums of 0/1 are exact ints)
                    for me, act, off in ((me1, s1, 6), (me2, s2, 2)):
                        Rr = me.shape[1]
                        nc.vector.tensor_tensor(
                            out=me[:, :, :], in0=me[:, :, :],
                            in1=act[:, off:off + Rr, :], op=ALU.add)

                    # ---- d3: up2(s3) conv + skip2(me2). K=128, psum col 0.
                    dec_layer(s3, d3, md3, w_d3s, 17, 32, 64, 0,
                              skip_wt=wsl['w_sk2'][:, 64 * t:64 * t + 64],
                              skip_src=me2, skip_trow=64,
                              last_mem=last, shadow=d3s)
                    # zero phantom d3/d3s row (buffer row 33) for d2's edge
                    # rows; twins are adjacent strips, one 128-wide op.
                    nc.vector.tensor_tensor(
                        out=d3k[:, 33:34, :], in0=d3k[:, 33:34, :],
                        in1=edge_mask[0:128, 0:66].rearrange(
                            "p (r w) -> p r w", w=66),
                        op=ALU.mult)
                    # ---- d2: up2(d3) conv + skip1(me1). K=64 base 0, psum col 96.
                    dec_d2(d3k, d2, md2b, wsl['w_d2s'],
                           skip_wt=wsl['w_sk1'][:, 32 * t:32 * t + 32],
                           skip_src=me1,
                           last_mem=last, shadow=d2s)
                    # zero phantom d2/d2s row (buffer row 65) for d1's edge
                    # rows; twins are adjacent strips, one 64-wide op.
                    nc.vector.tensor_tensor(
                        out=d2k[:, 65:66, :], in0=d2k[:, 65:66, :],
                        in1=edge_mask[64:128, 0:130].rearrange(
                            "p (r w) -> p r w", w=130),
                        op=ALU.mult)

                    # ---- d1: up2(d2) conv; 4 row-bands on 4 partition strips of md1,
                    #      4 col-concurrent matmuls per tap, 128-wide DVE ops.
                    #      band g strip 32g handles phase-rows k = o[g]+dlt, dlt in [0,18);
                    #      md1/d1scr row lr = i0 + 2*dlt; d1 buffer row = lr + 2*o[g].
                    o4 = (0, 15, 31, 47)
                    dblocks = [(0, 4), (4, 4), (8, 4), (12, 4), (16, 2)]
                    for pr in range(2):
                        i0 = 1 - pr
                        for pc in range(2):
                            for d0, nd in dblocks:
                                ps = pp.tile([128, 512], F32, name='ps', tag='ps')
                                views = [ps[32 * g:32 * g + 32, :nd * 128].rearrange(
                                    'p (r w) -> p r w', w=128) for g in range(4)]
                                for ab in range(2):
                                    wslice = wsl['w_d1s'][
                                        :, ((pr * 2 + pc) * 2 + ab) * 32:
                                        ((pr * 2 + pc) * 2 + ab + 1) * 32]
                                    for g in range(4):
                                        k0 = o4[g] + d0
                                        rhs = d2k[:, k0 + ab: k0 + ab + nd,
                                                  pc: pc + 128]
                                        nc.tensor.matmul(views[g], wslice, rhs,
                                                         start=(ab == 0), stop=(ab == 1),
                                                         tile_position=(64, 32 * g))
                                lr0 = i0 + 2 * d0
                                mrows = slice(lr0, lr0 + 2 * (nd - 1) + 1, 2)
                                mcols = slice(pc, pc + 2 * 127 + 1, 2)
                                nc.vector.scalar_tensor_tensor(
                                    out=md1[:, mrows, mcols], in0=md1[:, mrows, mcols],
                                    scalar=-0.5,
                                    in1=ps[0:128, :nd * 128].rearrange(
                                        'p (r w) -> p r w', w=128),
                                    op0=ALU.mult, op1=ALU.add)
                            urows = slice(i0, i0 + 2 * 17 + 1, 2)
                            mcols = slice(pc, pc + 2 * 127 + 1, 2)
                            scols = slice(1 + pc, 1 + pc + 2 * 127 + 1, 2)
                            if last:
                                # d1 spikes are only consumed by the flow conv
                                # at the final step; the reset recomputes is_gt
                                # itself, so skip the scratch write before then.
                                nc.vector.tensor_scalar(
                                    out=d1scr[:, urows, scols],
                                    in0=md1[:, urows, mcols],
                                    scalar1=1.0, scalar2=None, op0=ALU.is_gt)
                            if not last:
                                nc.vector.scalar_tensor_tensor(
                                    out=md1[:, urows, mcols], in0=md1[:, urows, mcols],
                                    scalar=1.0, in1=md1[:, urows, mcols],
                                    op0=ALU.is_gt, op1=ALU.subtract)
                    if last:
                        # zero phantom d1 buffer row 129 (strip-3 scratch row 35) on
                        # bottom-half cores: below-image zero-pad for the flow conv.
                        nc.vector.tensor_tensor(
                            out=d1scr[96:128, 35:36, :], in0=d1scr[96:128, 35:36, :],
                            in1=mask_bf[96:128, 0:258].rearrange('p (r w) -> p r w', w=258),
                            op=ALU.mult)
                        # ---- flow conv: chunk g (32 rows) from scratch strip g (bf16).
                        for g in range(4):
                            F0 = (0, 32, 64, 96)[g]
                            blocks = [(F0 + 8 * q + 2 * v, 2) for q in range(4) for v in range(4)]
                            for rs in range(0, len(blocks), 4):
                                rnd = blocks[rs:rs + 4]
                                ps = pp.tile([128, 512], F32, name='psf', tag='ps')
                                views = [ps[32 * j:32 * j + 2, :nr * 256].rearrange(
                                    'p (r w) -> p r w', w=256) for j, (f, nr) in enumerate(rnd)]
                                for i, (ky, kx) in enumerate(taps9):
                                    for j, (f, nr) in enumerate(rnd):
                                        sr = f + ky - 2 * (0, 15, 31, 47)[g]
                                        rhs = d1scr[32 * g:32 * g + 32, sr:sr + nr, kx:kx + 256]
                                        nc.tensor.matmul(
                                            views[j], tFw[32 * g:32 * g + 32, i * 2:(i + 1) * 2],
                                            rhs, start=(i == 0), stop=(i == 8),
                                            tile_position=(32 * g, 32 * j))
                                # stage psum->sbuf on the otherwise-idle ACT
                                # engine; 2 slots so DMAs pairwise overlap.
                                for j, (f, nr) in enumerate(rnd):
                                    sl = 2 * (j % 2)
                                    fsl = floscr[:, sl:sl + nr, :]
                                    nc.scalar.copy(fsl, views[j])
                                    nc.sync.dma_start(
                                        out=flow_d[f:f + nr].rearrange('r p w -> p r w'),
                                        in_=fsl)
                    if debug and rep == 0 and t == DBG_STEP:
                        for nm, v in (('s1', s1), ('s2', s2), ('s3', s3),
                                      ('d3', d3), ('d2', d2), ('me1', me1),
                                      ('me2', me2), ('m2', m2)):
                            nc.sync.dma_start(
                                out=dbg_d[nm][:],
                                in_=v.rearrange("p r w -> p (r w)"))
    return nc


DBG_STEP = 1


def make_in_maps(inputs):
    wpack = pack_weights(inputs)
    x = np.asarray(inputs['x'], np.float32)
    maps = []
    for core in range(8):
        n, h = core // 2, core % 2
        m = dict(wpack)
        m['x_e1'] = pack_x_core(x[n], 128 * h)
        m['edge_mask'] = np.full((128, 258), 1.0 - h, np.float32)
        m['edge_mask_bf'] = np.full((128, 258), 1.0 - h, ml_dtypes.bfloat16)
        maps.append(m)
    return maps


def assemble(results):
    out = np.zeros((4, 2, 256, 256), np.float32)
    for core in range(8):
        n, h = core // 2, core % 2
        out[n, :, 128 * h:128 * h + 128, :] = \
            results[core]["flow"].reshape(128, 2, 256).transpose(1, 0, 2)
    return out


# ---------------------------------------------------------------- entry point

def _split_waits(nc, max_waits=1):
    """Walrus here only fits one sem-wait slot per instruction; hoist excess
    waits onto same-engine no-ops inserted right before the instruction."""
    fn = nc.m.functions[0]
    n_new = 0
    for bb in fn.blocks:
        out = []
        for inst in bb.instructions:
            si = inst.sync_info
            if si is not None and si.on_wait and len(si.on_wait) > max_waits:
                waits = list(si.on_wait)
                keep = waits[-max_waits:]
                extra = waits[:-max_waits]
                for i in range(0, len(extra), max_waits):
                    chunk = extra[i:i + max_waits]
                    nop = mybir.InstNoOp(
                        name=nc.get_next_instruction_name(),
                        sync_info=mybir.SyncInfo(on_wait=list(chunk), on_update=[]),
                        bass_nofuse=True, engine=inst.engine, text_hint="waitfix")
                    nc.register_instruction(nop)
                    out.append(nop)
                    n_new += 1
                si.on_wait = keep
            out.append(inst)
        bb.instructions = out
    return n_new


_CACHED = {}


def kernel(**inputs):
    """Full-input entry: shards across 8 NeuronCores internally."""
    from concourse.bass_utils import run_bass_kernel_spmd
    if 'nc' not in _CACHED:
        nc = build_kernel(repeats=1, debug=False)
        _split_waits(nc, max_waits=1)
        _CACHED['nc'] = nc
    nc = _CACHED['nc']
    in_maps = make_in_maps(inputs)
    res = run_bass_kernel_spmd(nc, in_maps, list(range(8)))
    return assemble(res.results)

